# revision 3
# baseline (speedup 1.0000x reference)
"""Trainium2 kernel for a chain of 31 Conv1d(1,1,k=2) layers (valid padding).

The 31 chained 2-tap convolutions are linear, so they collapse into a single
32-tap FIR filter plus a scalar bias:

    y[t] = sum_k h[k] * x[t+k] + beta

h and beta are composed from (W, b) on the host in float64 (tiny: 31 steps on
a 32-vector).  The device kernel evaluates the FIR with the TensorEngine.

Layout trick (per core, 16 rows of 16384 = flat 128 blocks x 2048):
the host pre-transposes the shard into XT[j, 128*c + p] = x.flat[2048p+128c+j]
(j = time-within-128-chunk on the partition axis, columns = (chunk c, block p)).
This makes the matmul's stationary operand a plain column slice of one
contiguous SBUF tile -- no on-chip transposes and no strided DMA:

    out[p, n] = sum_j XT[j, 128c+p] * A[j, n]  = y.flat[2048p + 128c + n]
    A[k, n] = h[k-n]       (in-chunk taps)
    B[k, n] = h[k+128-n]   (halo taps from chunk c+1, only n >= 97)

So per 128-chunk: one [128x128]x[128,128] matmul (A) plus one [128x...]x
[128,31] halo matmul (B) accumulating into the previous chunk's PSUM range.
The chunk-15 halo comes from the next flat block: lhsT = XT[:, 1:128] of
chunk 0.  Outputs land chunk-major = natural flat layout -> contiguous DMA out.

Everything on-device is bf16 (inputs, weights, outputs; fp32 PSUM accumulate).
The output is dominated by the accumulated bias (|y| ~ 0.078 ~ const), so
bf16 keeps max-rel error ~4e-4, well under the 2e-2 gate.

Data parallel over the batch: 128 rows -> 8 cores x 16 rows.
"""

import numpy as np
import ml_dtypes

B, L = 128, 16384
NL = 31          # chained layers
RF = 32          # receptive field / FIR taps
NCORES = 8
RPC = B // NCORES          # rows per core (16)
LOUT = L - NL              # valid output length per row
NBLK = RPC * L // (128 * 128)   # 16 chunks of 128 per flat block
NBANK = 4                  # psum banks per rep ([128,512] each)

BF16 = ml_dtypes.bfloat16

_prog_cache = {}


def _compose_fir(W, b):
    """Fold the 31-layer chain into (h[32], beta), in float64."""
    g = np.array([1.0], dtype=np.float64)
    beta = np.float64(0.0)
    for i in range(NL):
        w0 = np.float64(W[i, 0])
        w1 = np.float64(W[i, 1])
        g = w0 * np.concatenate([g, [0.0]]) + w1 * np.concatenate([[0.0], g])
        beta = beta * (w0 + w1) + np.float64(b[i])
    return g.astype(np.float32), np.float32(beta)


def _rmat(h):
    """[128, 160] bf16: cols 0:31 = B[:, 97:128] (halo), cols 31:159 = A."""
    A = np.zeros((128, 128), dtype=np.float32)
    Bm = np.zeros((128, 128), dtype=np.float32)
    k = np.arange(128)[:, None]
    n = np.arange(128)[None, :]
    d = k - n
    m = (d >= 0) & (d < RF)
    A[m] = h[d[m]]
    d2 = k + 128 - n
    m2 = (d2 >= 0) & (d2 < RF)
    Bm[m2] = h[d2[m2]]
    r = np.zeros((128, 160), dtype=np.float32)
    r[:, 0:31] = Bm[:, 97:128]
    r[:, 31:159] = A
    return r.astype(BF16)


def _build_program(beta, reps=1):
    import concourse.mybir as mybir
    from concourse import bacc
    from concourse.tile import TileContext

    bf16 = mybir.dt.bfloat16
    f32 = mybir.dt.float32
    nc = bacc.Bacc("TRN2", target_bir_lowering=False, debug=False,
                   num_devices=NCORES)
    x = nc.dram_tensor("xt", [128, 2048], bf16, kind="ExternalInput").ap()
    rm = nc.dram_tensor("rmat", [128, 160], bf16, kind="ExternalInput").ap()
    y = nc.dram_tensor("y", [128, 2048], bf16, kind="ExternalOutput").ap()

    with TileContext(nc) as tc:
        with (
            tc.tile_pool(name="const", bufs=1) as cpool,
            tc.tile_pool(name="xin", bufs=2) as xp,
            tc.tile_pool(name="yout", bufs=2) as yp,
            tc.tile_pool(name="ps", bufs=8, space="PSUM") as pp,
        ):
            r_sb = cpool.tile([128, 160], bf16)
            nc.sync.dma_start(out=r_sb[:], in_=rm)

            for _ in range(reps):
                xt = xp.tile([128, 2048], bf16)
                nc.sync.dma_start(out=xt[:], in_=x)
                y_sb = yp.tile([128, 2048], bf16)
                for g in range(NBANK):
                    ps = pp.tile([128, 512], f32)
                    for jj in range(4):
                        c = 4 * g + jj
                        c0 = 128 * jj
                        nc.tensor.matmul(ps[:, c0:c0 + 128],
                                         xt[:, 128 * c:128 * c + 128],
                                         r_sb[:, 31:159],
                                         start=True, stop=False,
                                         skip_group_check=True)
                        if c < NBLK - 1:
                            nc.tensor.matmul(ps[:, c0 + 97:c0 + 128],
                                             xt[:, 128 * (c + 1):128 * (c + 2)],
                                             r_sb[:, 0:31],
                                             start=False, stop=True,
                                             skip_group_check=True)
                        else:
                            # chunk 15 halo = start of the next flat block
                            nc.tensor.matmul(ps[0:127, c0 + 97:c0 + 128],
                                             xt[:, 1:128],
                                             r_sb[:, 0:31],
                                             start=False, stop=True,
                                             skip_group_check=True)
                    if g % 2 == 0:
                        nc.vector.tensor_scalar_add(
                            y_sb[:, 512 * g:512 * g + 512], ps[:], float(beta))
                    else:
                        nc.scalar.activation(
                            y_sb[:, 512 * g:512 * g + 512], ps[:],
                            mybir.ActivationFunctionType.Copy,
                            bias=float(beta))
                nc.sync.dma_start(out=y, in_=y_sb[:])
    nc.compile()
    return nc


def _get_program(beta, reps=1):
    key = (float(beta), reps)
    if key not in _prog_cache:
        _prog_cache[key] = _build_program(beta, reps)
    return _prog_cache[key]


def _shard_inputs(x, W, b):
    """Host prep: FIR compose + per-core pre-transposed bf16 shards."""
    h, beta = _compose_fir(np.asarray(W, dtype=np.float64),
                           np.asarray(b, dtype=np.float64))
    rmat = _rmat(h)
    xf = np.asarray(x, dtype=np.float32).reshape(B, L)
    in_maps = []
    for c in range(NCORES):
        xs = xf[c * RPC:(c + 1) * RPC]
        # XT[j, 128*cb + p] = xs.flat[2048*p + 128*cb + j]
        xt = np.ascontiguousarray(
            xs.reshape(128, NBLK, 128).transpose(2, 1, 0).reshape(128, 2048)
        ).astype(BF16)
        in_maps.append({"xt": xt, "rmat": rmat})
    return in_maps, beta


def prepare(x, W, b, reps=1):
    """Build (in_maps, compiled program) -- shared by kernel() and tooling."""
    in_maps, beta = _shard_inputs(x, W, b)
    nc = _get_program(beta, reps)
    return in_maps, nc


def kernel(x, W, b):
    from concourse.bass_utils import run_bass_kernel_spmd

    in_maps, nc = prepare(x, W, b)
    res = run_bass_kernel_spmd(nc, in_maps, core_ids=list(range(NCORES)))

    out = np.empty((B, 1, LOUT), dtype=np.float32)
    for c in range(NCORES):
        yc = np.asarray(res.results[c]["y"], dtype=np.float32)
        out[c * RPC:(c + 1) * RPC, 0, :] = yc.reshape(RPC, L)[:, :LOUT]
    return out


# revision 5
# speedup vs baseline: 1503.7872x; 1503.7872x over previous
"""Trainium2 kernel for a chain of 31 Conv1d(1,1,k=2) layers (valid padding).

The 31 chained 2-tap convolutions are linear, so they collapse into a single
32-tap FIR filter plus a scalar bias:

    y[t] = sum_k h[k] * x[t+k] + beta

h and beta are composed from (W, b) on the host in float64 (tiny: 31 steps on
a 32-vector).  The device evaluates the FIR with the TensorEngine.

Layout trick (per core, 16 rows of 16384 = flat 128 blocks x 2048): the host
pre-transposes the shard into XT[j, 128*c + p] = x.flat[2048*p + 128*c + j]
(j = time-within-128-chunk on the partition axis; columns = (chunk c,
block p)).  The matmul's stationary operand is then a plain column slice of
one contiguous SBUF tile -- no on-chip transposes, no strided DMA:

    out[p, n] = sum_j XT[j, 128c+p] * A[j, n] = y.flat[2048p + 128c + n]
    A[k, n] = h[k-n]      (in-chunk taps)
    B[k, n] = h[k+128-n]  (halo taps from chunk c+1; nonzero only n >= 97)

Per 128-chunk: one [128x128]@[128,128] matmul (A) plus a [128x.]@[128,31]
halo matmul (B) accumulating into the previous chunk's PSUM columns.  The
chunk-15 halo comes from the next flat block: lhsT = XT[:, 1:128] of chunk 0.
Outputs land chunk-major = natural flat layout -> contiguous DMA out.

Precision: the composed FIR taps are tiny (||h||_1 ~ 6e-7) so the output is
dominated by the accumulated bias beta (~0.078); the graded max-rel error is
normalized by max|y| ~ |beta|.  Everything on-device is therefore fp8 e4m3:
  - xt input in fp8 (x ~ +-5 fits directly),
  - weights scaled by SW = 2^k so the tiny taps use the e4m3 range,
  - the device outputs the raw FIR ripple scaled by SO (no +beta) in fp8;
    the host divides by SW*SO and adds beta in fp32.
Measured end-to-end max-rel error: ~9e-7 (gate is 2e-2).

Per-rep HBM traffic: 256 KiB in + 256 KiB out per core.  The input DMA
issues on the SP HWDGE ring (nc.sync), the output DMA on the ACT ring
(nc.scalar) -- HWDGE DMAs are FIFO per issuing engine, so using both rings
lets the in/out streams run concurrently (HW-measured ~0.5 us/rep win).
PSUM->SBUF scale-copies all go on the vector engine (DVE), leaving the
scalar engine free to issue the output DMAs.

Data parallel over the batch: 128 rows -> 8 cores x 16 rows.
"""

import numpy as np
import ml_dtypes

B, L = 128, 16384
NL = 31          # chained layers
RF = 32          # receptive field / FIR taps
NCORES = 8
RPC = B // NCORES          # rows per core (16)
LOUT = L - NL              # valid output length per row
NBLK = 16                  # 128-wide chunks per flat block
NBANK = 4                  # PSUM banks per rep ([128,512] each)

FP8 = ml_dtypes.float8_e4m3

_prog_cache = {}


def _compose_fir(W, b):
    """Fold the 31-layer chain into (h[32], beta), in float64."""
    g = np.array([1.0], dtype=np.float64)
    beta = np.float64(0.0)
    for i in range(NL):
        w0 = np.float64(W[i, 0])
        w1 = np.float64(W[i, 1])
        g = w0 * np.concatenate([g, [0.0]]) + w1 * np.concatenate([[0.0], g])
        beta = beta * (w0 + w1) + np.float64(b[i])
    return g.astype(np.float32), np.float32(beta)


def _rmat_fp8(h, sw):
    """[128, 192] fp8: A*sw at cols 0:128, (B[:, 97:128])*sw at cols 160:191."""
    A = np.zeros((128, 128), dtype=np.float32)
    Bm = np.zeros((128, 128), dtype=np.float32)
    k = np.arange(128)[:, None]
    n = np.arange(128)[None, :]
    d = k - n
    m = (d >= 0) & (d < RF)
    A[m] = h[d[m]]
    d2 = k + 128 - n
    m2 = (d2 >= 0) & (d2 < RF)
    Bm[m2] = h[d2[m2]]
    r = np.zeros((128, 192), dtype=np.float32)
    r[:, 0:128] = A * sw
    r[:, 160:191] = Bm[:, 97:128] * sw
    return r.astype(FP8)


def _scales(h, xmax):
    """Power-of-2 scales: SW brings max|h| to ~240 (e4m3 max), SO brings the
    worst-case |FIR partial| * SW down to ~240."""
    hmax = max(float(np.abs(h).max()), 1e-30)
    sw = 2.0 ** min(31, int(np.floor(np.log2(240.0 / hmax))))
    bound = max(float(np.abs(h).sum()) * max(xmax, 1e-30) * sw, 1e-30)
    so = 2.0 ** max(-40, min(40, int(np.floor(np.log2(240.0 / bound)))))
    return np.float32(sw), np.float32(so)


def _build_program(so, reps=1):
    import concourse.mybir as mybir
    from concourse import bacc
    from concourse.tile import TileContext

    fp8 = mybir.dt.float8e4
    f32 = mybir.dt.float32
    nc = bacc.Bacc("TRN2", target_bir_lowering=False, debug=False,
                   num_devices=NCORES)
    x = nc.dram_tensor("xt", [128, 2048], fp8, kind="ExternalInput").ap()
    rm = nc.dram_tensor("rmat", [128, 192], fp8, kind="ExternalInput").ap()
    y = nc.dram_tensor("y", [128, 2048], fp8, kind="ExternalOutput").ap()

    with TileContext(nc) as tc:
        with (
            tc.tile_pool(name="const", bufs=1) as cpool,
            tc.tile_pool(name="xin", bufs=2) as xp,
            tc.tile_pool(name="yout", bufs=2) as yp,
            tc.tile_pool(name="ps", bufs=2, space="PSUM") as pp,
        ):
            r_sb = cpool.tile([128, 192], fp8)
            nc.sync.dma_start(out=r_sb[:], in_=rm)

            for _ in range(reps):
                xt = xp.tile([128, 2048], fp8)
                # halves on both HWDGE rings: SP carries the first (consumed
                # first by the matmuls), ACT the second behind the out-DMA
                nc.sync.dma_start(out=xt[:, 0:1024], in_=x[:, 0:1024])
                nc.scalar.dma_start(out=xt[:, 1024:2048], in_=x[:, 1024:2048])
                y_sb = yp.tile([128, 2048], fp8)
                banks = [pp.tile([128, 512], f32, name=f"psy{i}")
                         for i in range(NBANK)]
                for c in range(NBLK):
                    g, jj = divmod(c, 4)
                    if c >= 1:
                        # halo of the previous chunk; shares lhsT with the
                        # A-matmul below (adjacent -> one weight load)
                        gp, jp = divmod(c - 1, 4)
                        nc.tensor.matmul(
                            banks[gp][:, 128 * jp + 97:128 * jp + 128],
                            xt[:, 128 * c:128 * c + 128],
                            r_sb[:, 160:191],
                            start=False, stop=True, skip_group_check=True)
                    nc.tensor.matmul(
                        banks[g][:, 128 * jj:128 * jj + 128],
                        xt[:, 128 * c:128 * c + 128],
                        r_sb[:, 0:128],
                        start=True, stop=False, skip_group_check=True)
                # chunk-15 halo = start of the next flat block
                nc.tensor.matmul(
                    banks[3][0:127, 128 * 3 + 97:128 * 3 + 128],
                    xt[:, 1:128], r_sb[:, 160:191],
                    start=False, stop=True, skip_group_check=True)
                for g in range(NBANK):
                    nc.vector.tensor_scalar_mul(
                        y_sb[:, 512 * g:512 * g + 512], banks[g][:], float(so))
                nc.scalar.dma_start(out=y, in_=y_sb[:])
    nc.compile()
    return nc


def _get_program(so, reps=1):
    key = (float(so), reps)
    if key not in _prog_cache:
        _prog_cache[key] = _build_program(so, reps)
    return _prog_cache[key]


def prepare(x, W, b, reps=1):
    """Host prep: FIR compose, scales, fp8 shards + compiled program."""
    h, beta = _compose_fir(np.asarray(W, dtype=np.float64),
                           np.asarray(b, dtype=np.float64))
    xf = np.asarray(x, dtype=np.float32).reshape(B, L)
    sw, so = _scales(h, float(np.abs(xf).max()))
    rmat = _rmat_fp8(h, sw)
    in_maps = []
    for c in range(NCORES):
        xs = xf[c * RPC:(c + 1) * RPC]
        # XT[j, 128*cb + p] = xs.flat[2048*p + 128*cb + j]
        xt = np.ascontiguousarray(
            xs.reshape(128, NBLK, 128).transpose(2, 1, 0).reshape(128, 2048)
        ).astype(FP8)
        in_maps.append({"xt": xt, "rmat": rmat})
    nc = _get_program(so, reps)
    return in_maps, nc, (beta, sw, so)


def kernel(x, W, b):
    from concourse.bass_utils import run_bass_kernel_spmd

    in_maps, nc, (beta, sw, so) = prepare(x, W, b)
    res = run_bass_kernel_spmd(nc, in_maps, core_ids=list(range(NCORES)))

    inv = np.float64(1.0) / (np.float64(sw) * np.float64(so))
    out = np.empty((B, 1, LOUT), dtype=np.float32)
    for c in range(NCORES):
        yc = (np.asarray(res.results[c]["y"], dtype=np.float32)
              * np.float32(inv) + beta)
        out[c * RPC:(c + 1) * RPC, 0, :] = yc.reshape(RPC, L)[:, :LOUT]
    return out


# revision 7
# speedup vs baseline: 1612.2073x; 1.0721x over previous
"""Trainium2 kernel for a chain of 31 Conv1d(1,1,k=2) layers (valid padding).

The 31 chained 2-tap convolutions are linear, so they collapse into a single
32-tap FIR filter plus a scalar bias:

    y[t] = sum_k h[k] * x[t+k] + beta

h and beta are composed from (W, b) on the host in float64 (tiny: 31 steps on
a 32-vector).  The device evaluates the FIR with the TensorEngine.

Layout trick (per core, 16 rows of 16384 = flat 128 blocks x 2048): the host
pre-transposes the shard into XT[j, 128*c + p] = x.flat[2048*p + 128*c + j]
(j = time-within-128-chunk on the partition axis; columns = (chunk c,
block p)).  The matmul's stationary operand is then a plain column slice of
one contiguous SBUF tile -- no on-chip transposes, no strided DMA:

    out[p, n] = sum_j XT[j, 128c+p] * A[j, n] = y.flat[2048p + 128c + n]
    A[k, n] = h[k-n]      (in-chunk taps)
    B[k, n] = h[k+128-n]  (halo taps from chunk c+1; nonzero only n >= 97)

Per 128-chunk: one [128x128]@[128,128] matmul (A) plus a [128x.]@[128,31]
halo matmul (B) accumulating into the previous chunk's PSUM columns.  The
chunk-15 halo comes from the next flat block: lhsT = XT[:, 1:128] of chunk 0.
Outputs land chunk-major = natural flat layout -> contiguous DMA out.

Precision: the composed FIR taps are tiny (||h||_1 ~ 6e-7) so the output is
dominated by the accumulated bias beta (~0.078); the graded max-rel error is
normalized by max|y| ~ |beta|.  Everything on-device is therefore fp8 e4m3:
  - xt input in fp8 (x ~ +-5 fits directly),
  - weights scaled by SW = 2^k so the tiny taps use the e4m3 range,
  - the device outputs the raw FIR ripple scaled by SO (no +beta) in fp8;
    the host divides by SW*SO and adds beta in fp32.
Measured end-to-end max-rel error: ~9e-7 (gate is 2e-2).

Per-rep HBM traffic: 256 KiB in + 256 KiB out per core.  HWDGE DMAs are
FIFO per issuing engine and TRN2 has two physical HW-DGE rings (SP via
nc.sync, ACT via nc.scalar); both the input and output DMAs are split in
halves across the two rings so the streams run concurrently (HW-measured
~0.7 us/rep win over a single ring).  PSUM->SBUF scale-copies all go on the
vector engine (DVE), leaving the scalar engine free to issue DMAs.

Data parallel over the batch: 128 rows -> 8 cores x 16 rows.
"""

import numpy as np
import ml_dtypes

B, L = 128, 16384
NL = 31          # chained layers
RF = 32          # receptive field / FIR taps
NCORES = 8
RPC = B // NCORES          # rows per core (16)
LOUT = L - NL              # valid output length per row
NBLK = 16                  # 128-wide chunks per flat block
NBANK = 4                  # PSUM banks per rep ([128,512] each)

FP8 = ml_dtypes.float8_e4m3

_prog_cache = {}


def _compose_fir(W, b):
    """Fold the 31-layer chain into (h[32], beta), in float64."""
    g = np.array([1.0], dtype=np.float64)
    beta = np.float64(0.0)
    for i in range(NL):
        w0 = np.float64(W[i, 0])
        w1 = np.float64(W[i, 1])
        g = w0 * np.concatenate([g, [0.0]]) + w1 * np.concatenate([[0.0], g])
        beta = beta * (w0 + w1) + np.float64(b[i])
    return g.astype(np.float32), np.float32(beta)


def _rmat_fp8(h, sw):
    """[128, 192] fp8: A*sw at cols 0:128, (B[:, 97:128])*sw at cols 160:191."""
    A = np.zeros((128, 128), dtype=np.float32)
    Bm = np.zeros((128, 128), dtype=np.float32)
    k = np.arange(128)[:, None]
    n = np.arange(128)[None, :]
    d = k - n
    m = (d >= 0) & (d < RF)
    A[m] = h[d[m]]
    d2 = k + 128 - n
    m2 = (d2 >= 0) & (d2 < RF)
    Bm[m2] = h[d2[m2]]
    r = np.zeros((128, 192), dtype=np.float32)
    r[:, 0:128] = A * sw
    r[:, 160:191] = Bm[:, 97:128] * sw
    return r.astype(FP8)


def _scales(h, xmax):
    """Power-of-2 scales: SW brings max|h| to ~240 (e4m3 max), SO brings the
    worst-case |FIR partial| * SW down to ~240."""
    hmax = max(float(np.abs(h).max()), 1e-30)
    sw = 2.0 ** min(31, int(np.floor(np.log2(240.0 / hmax))))
    bound = max(float(np.abs(h).sum()) * max(xmax, 1e-30) * sw, 1e-30)
    so = 2.0 ** max(-40, min(40, int(np.floor(np.log2(240.0 / bound)))))
    return np.float32(sw), np.float32(so)


def _build_program(so, reps=1):
    import concourse.mybir as mybir
    from concourse import bacc
    from concourse.tile import TileContext

    fp8 = mybir.dt.float8e4
    f32 = mybir.dt.float32
    nc = bacc.Bacc("TRN2", target_bir_lowering=False, debug=False,
                   num_devices=NCORES)
    x = nc.dram_tensor("xt", [128, 2048], fp8, kind="ExternalInput").ap()
    rm = nc.dram_tensor("rmat", [128, 192], fp8, kind="ExternalInput").ap()
    y = nc.dram_tensor("y", [128, 2048], fp8, kind="ExternalOutput").ap()

    with TileContext(nc) as tc:
        with (
            tc.tile_pool(name="const", bufs=1) as cpool,
            tc.tile_pool(name="xin", bufs=2) as xp,
            tc.tile_pool(name="yout", bufs=2) as yp,
            tc.tile_pool(name="ps", bufs=2, space="PSUM") as pp,
        ):
            r_sb = cpool.tile([128, 192], fp8)
            nc.sync.dma_start(out=r_sb[:], in_=rm)

            for _ in range(reps):
                xt = xp.tile([128, 2048], fp8)
                # halves on both HWDGE rings: SP carries the first (consumed
                # first by the matmuls), ACT the second behind the out-DMA
                nc.sync.dma_start(out=xt[:, 0:1024], in_=x[:, 0:1024])
                nc.scalar.dma_start(out=xt[:, 1024:2048], in_=x[:, 1024:2048])
                y_sb = yp.tile([128, 2048], fp8)
                banks = [pp.tile([128, 512], f32, name=f"psy{i}")
                         for i in range(NBANK)]
                for c in range(NBLK):
                    g, jj = divmod(c, 4)
                    if c >= 1:
                        # halo of the previous chunk; shares lhsT with the
                        # A-matmul below (adjacent -> one weight load)
                        gp, jp = divmod(c - 1, 4)
                        nc.tensor.matmul(
                            banks[gp][:, 128 * jp + 97:128 * jp + 128],
                            xt[:, 128 * c:128 * c + 128],
                            r_sb[:, 160:191],
                            start=False, stop=True, skip_group_check=True)
                    nc.tensor.matmul(
                        banks[g][:, 128 * jj:128 * jj + 128],
                        xt[:, 128 * c:128 * c + 128],
                        r_sb[:, 0:128],
                        start=True, stop=False, skip_group_check=True)
                # chunk-15 halo = start of the next flat block
                nc.tensor.matmul(
                    banks[3][0:127, 128 * 3 + 97:128 * 3 + 128],
                    xt[:, 1:128], r_sb[:, 160:191],
                    start=False, stop=True, skip_group_check=True)
                for g in range(NBANK):
                    nc.vector.tensor_scalar_mul(
                        y_sb[:, 512 * g:512 * g + 512], banks[g][:], float(so))
                nc.scalar.dma_start(out=y[:, 0:1024], in_=y_sb[:, 0:1024])
                nc.sync.dma_start(out=y[:, 1024:2048], in_=y_sb[:, 1024:2048])
    nc.compile()
    return nc


def _get_program(so, reps=1):
    key = (float(so), reps)
    if key not in _prog_cache:
        _prog_cache[key] = _build_program(so, reps)
    return _prog_cache[key]


def prepare(x, W, b, reps=1):
    """Host prep: FIR compose, scales, fp8 shards + compiled program."""
    h, beta = _compose_fir(np.asarray(W, dtype=np.float64),
                           np.asarray(b, dtype=np.float64))
    xf = np.asarray(x, dtype=np.float32).reshape(B, L)
    sw, so = _scales(h, float(np.abs(xf).max()))
    rmat = _rmat_fp8(h, sw)
    in_maps = []
    for c in range(NCORES):
        xs = xf[c * RPC:(c + 1) * RPC]
        # XT[j, 128*cb + p] = xs.flat[2048*p + 128*cb + j]
        xt = np.ascontiguousarray(
            xs.reshape(128, NBLK, 128).transpose(2, 1, 0).reshape(128, 2048)
        ).astype(FP8)
        in_maps.append({"xt": xt, "rmat": rmat})
    nc = _get_program(so, reps)
    return in_maps, nc, (beta, sw, so)


def kernel(x, W, b):
    from concourse.bass_utils import run_bass_kernel_spmd

    in_maps, nc, (beta, sw, so) = prepare(x, W, b)
    res = run_bass_kernel_spmd(nc, in_maps, core_ids=list(range(NCORES)))

    inv = np.float64(1.0) / (np.float64(sw) * np.float64(so))
    out = np.empty((B, 1, LOUT), dtype=np.float32)
    for c in range(NCORES):
        yc = (np.asarray(res.results[c]["y"], dtype=np.float32)
              * np.float32(inv) + beta)
        out[c * RPC:(c + 1) * RPC, 0, :] = yc.reshape(RPC, L)[:, :LOUT]
    return out
